# revision 1
# baseline (speedup 1.0000x reference)
"""TRN2 Bass kernel for nn_IrrepsLinear (e3nn-style per-irrep linear layer).

Computation (per node n, N=200000 nodes, 480 features):
  out0 = (x0 @ W0 + modal_attr[batch[n]] @ W0m) / sqrt(130)   cols   0:128
  out1 = einsum('nim,io->nom', x1, W1) / sqrt(64)             cols 128:320
  out2 = einsum('nim,io->nom', x2, W2) / sqrt(32)             cols 320:480

Strategy: data-parallel over nodes across 8 NeuronCores (25000 nodes/core,
padded to 25088 = 49 macro-tiles of 512 nodes). The per-irrep einsums fold
into one 480x480 block-diagonal weight Wfull (scales included), so the layer
is out = x @ Wfull plus a per-node modal gather realized on-device as a
one-hot matmul:
  modal contribution = onehot(batch) @ G,   G = modal_attr @ (W0m/sqrt(130))

The pipeline computes in fp16 (11-bit mantissa, ~5e-4 relative error - the
tensor engine runs 16-bit operands at 1 cycle/row vs 4 for fp32):
  - x shards are cast to fp16 and stored node-block-major [128, nb*480] on
    the host so every DMA row is a contiguous DRAM run
  - x loads ride the SWDGE queue, output stores the HWDGE queue (two DMA
    queues running concurrently)
  - PE transposes x blocks [128n, 128f] -> psum [128f, 128n] (fp16 1cyc/row)
  - ACT copies psum -> SBUF xT operand tiles
  - matmuls: lhsT = xT chunk, rhs = Wfull blocks (fp16), accumulated into
    psum_o [128 nodes, 480] fp32 per node-block; PSUM freshness discipline:
    R2 (start=True, cols 128:480) -> 4 modal one-hot matmuls (cols 0:128,
    fresh overwrite) -> R0/R1/R3 accumulate
  - modal one-hot: batch ids broadcast via fp16 ones-matmul (exact for
    ids < 2048), 4 chunk compares on DVE (fp16 in, fp32 per-partition iota
    scalar), bf16/fp16 G matmuls
  - psum_o copied to an fp16 out tile (DVE + ACT), stored fp16, upcast on
    host during the inverse layout permutation
"""
import numpy as np

import concourse.bass as bass
import concourse.mybir as mybir
import concourse.tile as tile
from concourse import bacc
from concourse.bass_utils import run_bass_kernel_spmd
from concourse.masks import make_identity

f32 = mybir.dt.float32
f16 = mybir.dt.float16
i32 = mybir.dt.int32

N_CORES = 8
MUL0, MUL1, MUL2, NMOD = 128, 64, 32, 2
DTOT = 480
NODES_PER_MACRO = 512
NB = 4  # 128-node blocks per macro

CHUNKS = [(0, 128), (128, 256), (256, 384), (384, 480)]
# rhs blocks: (row range = f_in chunk, col range = f_out window)
RBLK = [((256, 384), (128, 480)),
        ((0, 128), (0, 128)),
        ((128, 256), (128, 320)),
        ((384, 480), (320, 480))]


def _build_wfull(W0, W1, W2):
    inv0 = np.float32(1.0) / np.sqrt(np.float32(MUL0 + NMOD))
    inv1 = np.float32(1.0) / np.sqrt(np.float32(MUL1))
    inv2 = np.float32(1.0) / np.sqrt(np.float32(MUL2))
    Wfull = np.zeros((DTOT, DTOT), dtype=np.float32)
    Wfull[0:128, 0:128] = W0 * inv0
    for m in range(3):
        Wfull[128 + m:320:3, 128 + m:320:3] = W1 * inv1
    for m in range(5):
        Wfull[320 + m:480:5, 320 + m:480:5] = W2 * inv2
    return Wfull


def _host_prep(x, modal_attr, W0, W0m, W1, W2, batch):
    x = np.asarray(x)
    N = x.shape[0]
    ns = N // N_CORES
    ns_pad = ((ns + NODES_PER_MACRO - 1) // NODES_PER_MACRO) * NODES_PER_MACRO
    Wfull = _build_wfull(np.asarray(W0, dtype=np.float32),
                         np.asarray(W1, dtype=np.float32),
                         np.asarray(W2, dtype=np.float32))
    inv0 = np.float32(1.0) / np.sqrt(np.float32(MUL0 + NMOD))
    w0m_s = (np.asarray(W0m, dtype=np.float32) * inv0).astype(np.float32)
    rblks = [np.ascontiguousarray(Wfull[r0:r1, c0:c1])
             for (r0, r1), (c0, c1) in RBLK]
    batch = np.asarray(batch)
    mattr = np.ascontiguousarray(np.asarray(modal_attr, dtype=np.float32))
    in_maps = []
    for i in range(N_CORES):
        xs = np.zeros((ns_pad, DTOT), dtype=np.float16)
        xs[:ns] = x[i * ns:(i + 1) * ns].astype(np.float16)
        # node-block-major: partition p holds nodes {128*nb + p} contiguously
        xs2 = np.ascontiguousarray(
            xs.reshape(-1, 128, DTOT).transpose(1, 0, 2).reshape(128, -1))
        bs = np.zeros((ns_pad,), dtype=np.float16)
        bs[:ns] = batch[i * ns:(i + 1) * ns].astype(np.float16)
        in_maps.append({
            "xs": xs2, "bs": bs,
            "r0": rblks[0], "r1": rblks[1], "r2": rblks[2], "r3": rblks[3],
            "w0m": w0m_s, "mattr": mattr,
        })
    return in_maps, ns, ns_pad


def _build_nc(ns_pad, macros_per_super=4):
    assert ns_pad % NODES_PER_MACRO == 0
    nmacro = ns_pad // NODES_PER_MACRO
    supers = []
    m0 = 0
    while m0 < nmacro:
        msup = min(macros_per_super, nmacro - m0)
        supers.append((m0, msup))
        m0 += msup
    nb_per_super = NB * macros_per_super
    nc = bacc.Bacc("TRN2", target_bir_lowering=False, debug=False)

    nbs_tot = ns_pad // 128
    xs = nc.dram_tensor("xs", [128, nbs_tot * DTOT], f16,
                        kind="ExternalInput").ap()
    bs = nc.dram_tensor("bs", [ns_pad], f16, kind="ExternalInput").ap()
    rdr = []
    for k, ((r0_, r1_), (c0_, c1_)) in enumerate(RBLK):
        rdr.append(nc.dram_tensor(f"r{k}", [r1_ - r0_, c1_ - c0_], f32,
                                  kind="ExternalInput").ap())
    w0m = nc.dram_tensor("w0m", [NMOD, 128], f32, kind="ExternalInput").ap()
    mattr = nc.dram_tensor("mattr", [512, NMOD], f32, kind="ExternalInput").ap()
    ys = nc.dram_tensor("ys", [128, nbs_tot * DTOT], f16,
                        kind="ExternalOutput").ap()

    with tile.TileContext(nc) as tc:
        with tc.tile_pool(name="const", bufs=1) as cpool, \
             tc.tile_pool(name="sb", bufs=3) as sb, \
             tc.tile_pool(name="sbx", bufs=4) as sbx, \
             tc.tile_pool(name="sbo", bufs=3) as sbo, \
             tc.tile_pool(name="psb", bufs=2, space="PSUM") as psb, \
             tc.tile_pool(name="ps3", bufs=3, space="PSUM") as ps3:

            # ---------------- setup ----------------
            ident = cpool.tile([128, 128], f32, tag="ident")
            make_identity(nc, ident[:])
            ident_16 = cpool.tile([128, 128], f16, tag="ident16")
            nc.vector.tensor_copy(ident_16[:], ident[:])

            ones16 = cpool.tile([1, 128], f16, tag="ones16")
            nc.gpsimd.memset(ones16[:], 1.0)

            pidxf = []
            for c in range(4):
                t_i = cpool.tile([128, 1], i32, tag=f"pidx{c}i")
                nc.gpsimd.iota(t_i[:], pattern=[[0, 1]], base=128 * c,
                               channel_multiplier=1)
                t_f = cpool.tile([128, 1], f32, tag=f"pidx{c}f")
                nc.vector.tensor_copy(t_f[:], t_i[:])
                pidxf.append(t_f)

            rfr = []
            for k in range(4):
                rows, cols = rdr[k].shape
                t0 = cpool.tile([rows, cols], f32, tag=f"r{k}raw")
                nc.sync.dma_start(out=t0[:], in_=rdr[k])
                t1 = cpool.tile([rows, cols], f16, tag=f"r{k}16")
                nc.vector.tensor_copy(t1[:], t0[:])
                rfr.append(t1)

            w0m_sb = cpool.tile([NMOD, 128], f32, tag="w0mraw")
            nc.sync.dma_start(out=w0m_sb[:], in_=w0m)
            w0m_16 = cpool.tile([NMOD, 128], f16, tag="w0m16")
            nc.vector.tensor_copy(w0m_16[:], w0m_sb[:])

            mattr_sb = cpool.tile([128, 4 * NMOD], f32, tag="mattrraw")
            nc.sync.dma_start(out=mattr_sb[:].rearrange("g (c j) -> g c j", c=4),
                              in_=mattr.rearrange("(c g) j -> g c j", c=4))
            ps_mat = ps3.tile([NMOD, 512], f32, tag="xt")
            for c in range(4):
                nc.tensor.matmul(ps_mat[:, 128 * c:128 * (c + 1)],
                                 mattr_sb[:, NMOD * c:NMOD * (c + 1)],
                                 ident[:], is_transpose=True,
                                 start=(c == 0), stop=(c == 3),
                                 skip_group_check=True)
            maT_16 = cpool.tile([NMOD, 512], f16, tag="maT16")
            nc.vector.tensor_copy(maT_16[:], ps_mat[:])
            gch = []
            for c in range(4):
                ps_g = ps3.tile([128, 128], f32, tag="po")
                nc.tensor.matmul(ps_g[:], maT_16[:, 128 * c:128 * (c + 1)],
                                 w0m_16[:], start=True, stop=True)
                g_c = cpool.tile([128, 128], f16, tag=f"g{c}")
                nc.vector.tensor_copy(g_c[:], ps_g[:])
                gch.append(g_c)

            # ---------------- main loop ----------------
            for m0, msup in supers:
                n0 = m0 * NODES_PER_MACRO
                x_sb = sbx.tile([128, nb_per_super * DTOT], f16, tag="x")
                h1 = (msup + 1) // 2
                col0 = m0 * NB * DTOT
                for lo, hi in ((0, h1), (h1, msup)):
                    if hi <= lo:
                        continue
                    # x loads on the SWDGE queue (output uses the HWDGE queue)
                    nc.gpsimd.dma_start(
                        out=x_sb[:, lo * NB * DTOT:hi * NB * DTOT],
                        in_=xs[:, col0 + lo * NB * DTOT:col0 + hi * NB * DTOT])

                bs_sb = sb.tile([1, NODES_PER_MACRO * macros_per_super], f16,
                                tag="bs")
                nodes = msup * NODES_PER_MACRO
                nc.sync.dma_start(
                    out=bs_sb[:, :nodes],
                    in_=bs[n0:n0 + nodes].rearrange("(one n) -> one n", one=1))

                out_sb = sbo.tile([128, nb_per_super * DTOT], f16, tag="out")

                for q in range(msup):
                    ps_bb = psb.tile([128, NODES_PER_MACRO], f32, tag="bb")
                    nc.tensor.matmul(
                        ps_bb[:], ones16[:],
                        bs_sb[:, NODES_PER_MACRO * q:NODES_PER_MACRO * (q + 1)],
                        start=True, stop=True)
                    bb_sb = sb.tile([128, NODES_PER_MACRO], f16, tag="bbs")
                    nc.vector.tensor_copy(bb_sb[:], ps_bb[:])

                    ohs = []
                    for c in range(4):
                        oh = sb.tile([128, NODES_PER_MACRO], f16, tag=f"oh{c}")
                        nc.vector.tensor_scalar(oh[:], bb_sb[:], pidxf[c][:],
                                                None,
                                                op0=mybir.AluOpType.is_equal)
                        ohs.append(oh)

                    for nbq in range(NB):
                        nb = NB * q + nbq
                        ps_xt = ps3.tile([128, 512], f16, tag="xt")
                        for c, (f0, f1) in enumerate(CHUNKS):
                            cp = f1 - f0
                            nc.tensor.matmul(
                                ps_xt[0:cp, 128 * c:128 * c + 128],
                                x_sb[:, DTOT * nb + f0:DTOT * nb + f1],
                                ident_16[:], is_transpose=True,
                                start=(c == 0), stop=(c == 3),
                                skip_group_check=True)
                        xt_sb = sb.tile([128, 512], f16, tag="xts")
                        nc.scalar.copy(xt_sb[:, 0:384], ps_xt[:, 0:384])
                        nc.scalar.copy(xt_sb[0:96, 384:512],
                                       ps_xt[0:96, 384:512])

                        ps_o = ps3.tile([128, DTOT], f32, tag="po")

                        def mm_rblk(k, start, stop):
                            (r0_, r1_), (c0_, c1_) = RBLK[k]
                            kp = r1_ - r0_
                            ch = r0_ // 128
                            nc.tensor.matmul(
                                ps_o[:, c0_:c1_],
                                xt_sb[0:kp, 128 * ch:128 * ch + 128],
                                rfr[k][:], start=start, stop=stop,
                                skip_group_check=True)

                        mm_rblk(0, True, False)
                        for c in range(4):
                            nc.tensor.matmul(
                                ps_o[:, 0:128],
                                ohs[c][:, 128 * nbq:128 * nbq + 128],
                                gch[c][:], start=False, stop=False,
                                skip_group_check=True)
                        mm_rblk(1, False, False)
                        mm_rblk(2, False, False)
                        mm_rblk(3, False, True)

                        if nbq == 3:
                            nc.scalar.copy(
                                out_sb[:, DTOT * nb:DTOT * (nb + 1)], ps_o[:])
                        else:
                            nc.vector.tensor_copy(
                                out_sb[:, DTOT * nb:DTOT * (nb + 1)], ps_o[:])

                for lo, hi in ((0, h1), (h1, msup)):
                    if hi <= lo:
                        continue
                    nc.sync.dma_start(
                        out=ys[:, col0 + lo * NB * DTOT:col0 + hi * NB * DTOT],
                        in_=out_sb[:, lo * NB * DTOT:hi * NB * DTOT])

    nc.compile()
    return nc


_NC_CACHE = {}


def kernel(x, modal_attr, W0, W0m, W1, W2, batch):
    in_maps, ns, ns_pad = _host_prep(x, modal_attr, W0, W0m, W1, W2, batch)
    if ns_pad not in _NC_CACHE:
        _NC_CACHE[ns_pad] = _build_nc(ns_pad)
    nc = _NC_CACHE[ns_pad]
    res = run_bass_kernel_spmd(nc, in_maps, core_ids=list(range(N_CORES)))
    nbs = ns_pad // 128
    outs = []
    for i in range(N_CORES):
        ys2 = res.results[i]["ys"]
        outs.append(ys2.reshape(128, nbs, DTOT).transpose(1, 0, 2)
                    .reshape(ns_pad, DTOT)[:ns].astype(np.float32))
    return np.ascontiguousarray(np.concatenate(outs, axis=0))



# revision 8
# speedup vs baseline: 1.4272x; 1.4272x over previous
"""TRN2 Bass kernel for nn_IrrepsLinear (e3nn-style per-irrep linear layer).

Computation (per node n, N=200000 nodes, 480 features):
  out0 = (x0 @ W0 + modal_attr[batch[n]] @ W0m) / sqrt(130)   cols   0:128
  out1 = einsum('nim,io->nom', x1, W1) / sqrt(64)             cols 128:320
  out2 = einsum('nim,io->nom', x2, W2) / sqrt(32)             cols 320:480

Strategy: data-parallel over nodes across 8 NeuronCores (25000 nodes/core,
padded to 25088 = 196 blocks of 128 nodes). All layout work happens on the
host so the device runs a pure streaming GEMM at the HBM roofline:

  - the modal gather modal_attr[batch] is a host-side table lookup; its two
    values are appended to x as input features 480:482, with W0m/sqrt(130)
    as the matching weight rows (the FLOPs stay on device)
  - the 1e/2e irreps are de-interleaved m-major on the host, which turns the
    480x480 block weight into a block-diagonal matrix whose blocks are all
    <=128 wide: diag(W0, W1, W1, W1, W2, W2, W2, W2, W2) -> input chunks
    c0=0:128, c1=128:256, c2=256:384 map to the same output column ranges
    and c3=384:480 (+modal rows) maps to cols 384:480 and 0:128
  - x shards are cast to fp16 and stored TRANSPOSED per 128-node block
    ([feature, node] tiles) so the device needs no PE transposes: chunk
    tiles feed the PE directly as the stationary operand
  - per block: 5 matmuls (weights are the moving operand, free dim <=128,
    ~608 PE columns total) accumulate into one PSUM tile [128 nodes, 480],
    which one copy (rotated across DVE/ACT/GpSimd) casts to an fp16 output
    tile; outputs are stored fp16 and upcast on the host
  - input DMAs ride the SWDGE queue (gpsimd), output stores the HWDGE
    queue (sync), both split in halves per 14-block super-tile so the
    queues stream continuously
"""
import numpy as np

import concourse.bass as bass
import concourse.mybir as mybir
import concourse.tile as tile
from concourse import bacc
from concourse.bass_utils import run_bass_kernel_spmd

f32 = mybir.dt.float32
f16 = mybir.dt.float16

N_CORES = 8
MUL0, MUL1, MUL2, NMOD = 128, 64, 32, 2
DTOT = 480
DAUG = 482          # 480 features + 2 modal values
BLK = 128           # nodes per block
SUP = 14            # blocks per super-tile


def _feature_perm():
    """std feature index for each de-interleaved (m-major) column."""
    p1 = (MUL0 + 3 * np.arange(MUL1)[None, :] + np.arange(3)[:, None]).reshape(-1)
    p2 = (MUL0 + 3 * MUL1 + 5 * np.arange(MUL2)[None, :]
          + np.arange(5)[:, None]).reshape(-1)
    return np.concatenate([np.arange(MUL0), p1, p2])


def _block_diag(*ms):
    n = sum(m.shape[0] for m in ms)
    out = np.zeros((n, n), dtype=ms[0].dtype)
    o = 0
    for m in ms:
        out[o:o + m.shape[0], o:o + m.shape[0]] = m
        o += m.shape[0]
    return out


def _host_prep(x, modal_attr, W0, W0m, W1, W2, batch):
    x = np.asarray(x)
    batch = np.asarray(batch)
    N = x.shape[0]
    ns = N // N_CORES
    nblk = (ns + BLK - 1) // BLK
    ns_pad = nblk * BLK

    inv0 = np.float32(1.0) / np.sqrt(np.float32(MUL0 + NMOD))
    inv1 = np.float32(1.0) / np.sqrt(np.float32(MUL1))
    inv2 = np.float32(1.0) / np.sqrt(np.float32(MUL2))
    W0s = (np.asarray(W0, np.float32) * inv0).astype(np.float16)
    W1s = (np.asarray(W1, np.float32) * inv1).astype(np.float16)
    W2s = (np.asarray(W2, np.float32) * inv2).astype(np.float16)
    r0 = np.ascontiguousarray(W0s)
    r1 = _block_diag(W1s, W1s)
    r2 = _block_diag(W1s, W2s, W2s)
    r3 = _block_diag(W2s, W2s, W2s)
    rm = (np.asarray(W0m, np.float32) * inv0).astype(np.float16)

    p = _feature_perm()
    modal = np.asarray(modal_attr, np.float32)[batch]  # [N, 2] host gather

    in_maps = []
    for i in range(N_CORES):
        xa = np.zeros((ns_pad, DAUG), dtype=np.float16)
        xa[:ns, :DTOT] = x[i * ns:(i + 1) * ns][:, p]
        xa[:ns, DTOT:] = modal[i * ns:(i + 1) * ns]
        # chunks 0-2 as [feature, node] tiles: A[p, (b*3+c)*128 + j]
        A = np.ascontiguousarray(
            xa[:, :384].reshape(nblk, BLK, 3, 128).transpose(3, 0, 2, 1)
        ).reshape(128, nblk * 384)
        # chunk 3 (96 x2 features): B[p, b*128 + j]; modal separate [2, n]
        B = np.ascontiguousarray(
            xa[:, 384:480].reshape(nblk, BLK, 96).transpose(2, 0, 1)
        ).reshape(96, nblk * BLK)
        Bm = np.ascontiguousarray(
            xa[:, 480:].reshape(nblk, BLK, 2).transpose(2, 0, 1)
        ).reshape(2, nblk * BLK)
        in_maps.append({"xsA": A, "xsB": B, "xsBm": Bm,
                        "r0": r0, "r1": r1, "r2": r2, "r3": r3, "rm": rm})
    return in_maps, ns, ns_pad


def _build_nc(ns_pad):
    nblk = ns_pad // BLK
    supers = []
    b0 = 0
    while b0 < nblk:
        supers.append((b0, min(SUP, nblk - b0)))
        b0 += SUP

    nc = bacc.Bacc("TRN2", target_bir_lowering=False, debug=False)
    xsA = nc.dram_tensor("xsA", [128, nblk * 384], f16, kind="ExternalInput").ap()
    xsB = nc.dram_tensor("xsB", [96, nblk * BLK], f16, kind="ExternalInput").ap()
    xsBm = nc.dram_tensor("xsBm", [2, nblk * BLK], f16, kind="ExternalInput").ap()
    rdr = [nc.dram_tensor(n, list(s), f16, kind="ExternalInput").ap()
           for n, s in (("r0", (128, 128)), ("r1", (128, 128)),
                        ("r2", (128, 128)), ("r3", (96, 96)), ("rm", (2, 128)))]
    ys = nc.dram_tensor("ys", [128, nblk * DTOT], f16, kind="ExternalOutput").ap()

    with tile.TileContext(nc) as tc:
        with tc.tile_pool(name="const", bufs=1) as cpool, \
             tc.tile_pool(name="sba", bufs=4) as sba, \
             tc.tile_pool(name="sbb", bufs=4) as sbb, \
             tc.tile_pool(name="sbo", bufs=3) as sbo, \
             tc.tile_pool(name="ps", bufs=6, space="PSUM") as psp:

            wt = []
            for k, dr in enumerate(rdr):
                t = cpool.tile(list(dr.shape), f16, tag=f"w{k}")
                nc.sync.dma_start(out=t[:], in_=dr)
                wt.append(t)
            w0, w1, w2, w3, wm = wt

            # whole modal stream resident in SBUF (2 partitions x ~50KB)
            bmod = cpool.tile([2, nblk * BLK], f16, tag="bmod")
            nc.sync.dma_start(out=bmod[:], in_=xsBm)

            for b0, sblk in supers:
                xA = sba.tile([128, SUP * 384], f16, tag="xa")
                xB = sbb.tile([96, SUP * BLK], f16, tag="xb")
                h = (sblk + 1) // 2
                for lo, hi in ((0, h), (h, sblk)):
                    if hi <= lo:
                        continue
                    nc.gpsimd.dma_start(
                        out=xA[:, lo * 384:hi * 384],
                        in_=xsA[:, (b0 + lo) * 384:(b0 + hi) * 384])
                    nc.gpsimd.dma_start(
                        out=xB[:, lo * BLK:hi * BLK],
                        in_=xsB[:, (b0 + lo) * BLK:(b0 + hi) * BLK])

                out_sb = sbo.tile([128, SUP * DTOT], f16, tag="out")

                for b in range(sblk):
                    ps_o = psp.tile([128, DTOT], f32, tag="po")
                    a0 = b * 384
                    nb = b * BLK
                    mm = nc.tensor.matmul
                    mm(ps_o[:, 0:128], xA[:, a0:a0 + 128], w0[:],
                       start=True, stop=False, skip_group_check=True)
                    mm(ps_o[:, 0:128], bmod[:, (b0 + b) * BLK:(b0 + b + 1) * BLK],
                       wm[:], start=False, stop=True, skip_group_check=True)
                    mm(ps_o[:, 128:256], xA[:, a0 + 128:a0 + 256], w1[:],
                       start=True, stop=True, skip_group_check=True)
                    mm(ps_o[:, 256:384], xA[:, a0 + 256:a0 + 384], w2[:],
                       start=True, stop=True, skip_group_check=True)
                    mm(ps_o[:, 384:480], xB[0:96, nb:nb + BLK], w3[:],
                       start=True, stop=True, skip_group_check=True)

                    o_sl = out_sb[:, b * DTOT:(b + 1) * DTOT]
                    if b % 2 == 0:
                        nc.vector.tensor_copy(o_sl, ps_o[:])
                    else:
                        nc.scalar.copy(o_sl, ps_o[:])

                for lo, hi in ((0, h), (h, sblk)):
                    if hi <= lo:
                        continue
                    nc.sync.dma_start(
                        out=ys[:, (b0 + lo) * DTOT:(b0 + hi) * DTOT],
                        in_=out_sb[:, lo * DTOT:hi * DTOT])

    nc.compile()
    return nc


_NC_CACHE = {}


def kernel(x, modal_attr, W0, W0m, W1, W2, batch):
    in_maps, ns, ns_pad = _host_prep(x, modal_attr, W0, W0m, W1, W2, batch)
    if ns_pad not in _NC_CACHE:
        _NC_CACHE[ns_pad] = _build_nc(ns_pad)
    nc = _NC_CACHE[ns_pad]
    res = run_bass_kernel_spmd(nc, in_maps, core_ids=list(range(N_CORES)))
    nblk = ns_pad // BLK
    p = _feature_perm()
    invp = np.empty(DTOT, dtype=np.int64)
    invp[p] = np.arange(DTOT)
    outs = []
    for i in range(N_CORES):
        ysd = res.results[i]["ys"].reshape(128, nblk, DTOT)
        o = ysd[:, :, invp].transpose(1, 0, 2).reshape(ns_pad, DTOT)[:ns]
        outs.append(o.astype(np.float32))
    return np.ascontiguousarray(np.concatenate(outs, axis=0))


# revision 10
# speedup vs baseline: 1.4924x; 1.0457x over previous
"""TRN2 Bass kernel for nn_IrrepsLinear (e3nn-style per-irrep linear layer).

Computation (per node n, N=200000 nodes, 480 features):
  out0 = (x0 @ W0 + modal_attr[batch[n]] @ W0m) / sqrt(130)   cols   0:128
  out1 = einsum('nim,io->nom', x1, W1) / sqrt(64)             cols 128:320
  out2 = einsum('nim,io->nom', x2, W2) / sqrt(32)             cols 320:480

Strategy: data-parallel over nodes across 8 NeuronCores (25000 nodes/core,
padded to 25088 = 196 blocks of 128 nodes). All layout work happens on the
host so the device runs a pure streaming GEMM at the HBM roofline:

  - the modal gather modal_attr[batch] is a host-side table lookup; its two
    values are appended to x as extra input features, with W0m/sqrt(130) as
    the matching weight rows (the FLOPs stay on device)
  - the 1e/2e irreps are de-interleaved m-major on the host, which turns the
    480x480 block weight into a block-diagonal matrix whose blocks are all
    <=128 wide: input chunks c0=x0, c1=(x1 m0,m1), c2=(x1 m2, x2 m0,m1),
    c3=(x2 m2,m3,m4 + modal, zero-padded to 128 rows)
  - x shards are cast to fp16 and stored TRANSPOSED per 128-node block
    ([feature, node] tiles) so the device needs no PE transposes: chunk
    tiles feed the PE directly as the stationary operand
  - per block: 4 matmuls, each a full 128-row stationary load (partial
    row-group loads stall the PE pipe), 608 moving columns total; outputs
    are computed in reordered columns [c1-out | c2-out | c3-out | c0-out]
    so chunk 3's rhs (x2 blocks + modal rows into out0) spans a contiguous
    224 columns; the host applies the inverse permutation
  - one PSUM tile [128 nodes, 480] per block, cast to fp16 by a copy
    alternating between DVE and ACT; outputs stored fp16, upcast on host
  - input DMAs ride the SWDGE queue (gpsimd), output stores the HWDGE
    queue (sync), both split in halves per 14-block super-tile so the
    queues stream continuously
"""
import numpy as np

import concourse.bass as bass
import concourse.mybir as mybir
import concourse.tile as tile
from concourse import bacc
from concourse.bass_utils import run_bass_kernel_spmd

f32 = mybir.dt.float32
f16 = mybir.dt.float16

N_CORES = 8
MUL0, MUL1, MUL2, NMOD = 128, 64, 32, 2
DTOT = 480
BLK = 128           # nodes per block
SUP = 14            # blocks per super-tile


def _in_perm():
    """std feature index for each de-interleaved (m-major) input column."""
    p1 = (MUL0 + 3 * np.arange(MUL1)[None, :] + np.arange(3)[:, None]).reshape(-1)
    p2 = (MUL0 + 3 * MUL1 + 5 * np.arange(MUL2)[None, :]
          + np.arange(5)[:, None]).reshape(-1)
    return np.concatenate([np.arange(MUL0), p1, p2])


def _out_perm():
    """std feature index for each device output column.

    Device column order: [c1-out (x1 m0,m1) | c2-out (x1 m2, x2 m0,m1) |
    c3-out (x2 m2,m3,m4) | c0-out (x0+modal)].
    """
    p = _in_perm()
    return np.concatenate([p[128:480], p[0:128]])


def _block_diag(*ms):
    n = sum(m.shape[0] for m in ms)
    out = np.zeros((n, n), dtype=ms[0].dtype)
    o = 0
    for m in ms:
        out[o:o + m.shape[0], o:o + m.shape[0]] = m
        o += m.shape[0]
    return out


def _host_prep(x, modal_attr, W0, W0m, W1, W2, batch):
    x = np.asarray(x)
    batch = np.asarray(batch)
    N = x.shape[0]
    ns = N // N_CORES
    nblk = (ns + BLK - 1) // BLK
    ns_pad = nblk * BLK

    inv0 = np.float32(1.0) / np.sqrt(np.float32(MUL0 + NMOD))
    inv1 = np.float32(1.0) / np.sqrt(np.float32(MUL1))
    inv2 = np.float32(1.0) / np.sqrt(np.float32(MUL2))
    W0s = (np.asarray(W0, np.float32) * inv0).astype(np.float16)
    W1s = (np.asarray(W1, np.float32) * inv1).astype(np.float16)
    W2s = (np.asarray(W2, np.float32) * inv2).astype(np.float16)
    r0 = np.ascontiguousarray(W0s)
    r1 = _block_diag(W1s, W1s)
    r2 = _block_diag(W1s, W2s, W2s)
    # chunk-3 rhs [128, 224]: x2 m2,m3,m4 -> cols 0:96, modal -> cols 96:224
    r3 = np.zeros((128, 224), dtype=np.float16)
    r3[0:96, 0:96] = _block_diag(W2s, W2s, W2s)
    r3[96:98, 96:224] = (np.asarray(W0m, np.float32) * inv0).astype(np.float16)

    p = _in_perm()
    modal = np.asarray(modal_attr, np.float32)[batch]  # [N, 2] host gather

    in_maps = []
    for i in range(N_CORES):
        xa = np.zeros((ns_pad, 482), dtype=np.float16)
        xa[:ns, :DTOT] = x[i * ns:(i + 1) * ns][:, p]
        xa[:ns, DTOT:] = modal[i * ns:(i + 1) * ns]
        # chunks 0-2 as [feature, node] tiles: A[p, (b*3+c)*128 + j]
        A = np.ascontiguousarray(
            xa[:, :384].reshape(nblk, BLK, 3, 128).transpose(3, 0, 2, 1)
        ).reshape(128, nblk * 384)
        # chunk 3 (96 x2 features + 2 modal): B[p, b*128 + j]
        B = np.ascontiguousarray(
            xa[:, 384:].reshape(nblk, BLK, 98).transpose(2, 0, 1)
        ).reshape(98, nblk * BLK)
        in_maps.append({"xsA": A, "xsB": B,
                        "r0": r0, "r1": r1, "r2": r2, "r3": r3})
    return in_maps, ns, ns_pad


def _build_nc(ns_pad):
    nblk = ns_pad // BLK
    supers = []
    b0 = 0
    while b0 < nblk:
        supers.append((b0, min(SUP, nblk - b0)))
        b0 += SUP

    nc = bacc.Bacc("TRN2", target_bir_lowering=False, debug=False)
    xsA = nc.dram_tensor("xsA", [128, nblk * 384], f16, kind="ExternalInput").ap()
    xsB = nc.dram_tensor("xsB", [98, nblk * BLK], f16, kind="ExternalInput").ap()
    rdr = [nc.dram_tensor(n, list(s), f16, kind="ExternalInput").ap()
           for n, s in (("r0", (128, 128)), ("r1", (128, 128)),
                        ("r2", (128, 128)), ("r3", (128, 224)))]
    ys = nc.dram_tensor("ys", [128, nblk * DTOT], f16, kind="ExternalOutput").ap()

    with tile.TileContext(nc) as tc:
        with tc.tile_pool(name="const", bufs=1) as cpool, \
             tc.tile_pool(name="sba", bufs=4) as sba, \
             tc.tile_pool(name="sbb", bufs=4) as sbb, \
             tc.tile_pool(name="sbo", bufs=3) as sbo, \
             tc.tile_pool(name="ps", bufs=6, space="PSUM") as psp:

            wt = []
            for k, dr in enumerate(rdr):
                t = cpool.tile(list(dr.shape), f16, tag=f"w{k}")
                nc.sync.dma_start(out=t[:], in_=dr)
                wt.append(t)
            w0, w1, w2, w3 = wt

            for b0, sblk in supers:
                xA = sba.tile([128, SUP * 384], f16, tag="xa")
                xB = sbb.tile([128, SUP * BLK], f16, tag="xb")
                # zero the K-padding rows once per super (32-aligned partition
                # base); the DMA then overwrites rows 0:98 with real data
                nc.gpsimd.memset(xB[96:128, :sblk * BLK], 0.0)
                h = (sblk + 1) // 2
                for lo, hi in ((0, h), (h, sblk)):
                    if hi <= lo:
                        continue
                    nc.gpsimd.dma_start(
                        out=xA[:, lo * 384:hi * 384],
                        in_=xsA[:, (b0 + lo) * 384:(b0 + hi) * 384])
                    nc.gpsimd.dma_start(
                        out=xB[0:98, lo * BLK:hi * BLK],
                        in_=xsB[:, (b0 + lo) * BLK:(b0 + hi) * BLK])

                out_sb = sbo.tile([128, SUP * DTOT], f16, tag="out")

                for b in range(sblk):
                    ps_o = psp.tile([128, DTOT], f32, tag="po")
                    a0 = b * 384
                    nb = b * BLK
                    mm = nc.tensor.matmul
                    # c3 (+modal rows): fresh write of cols 256:480
                    mm(ps_o[:, 256:480], xB[:, nb:nb + BLK], w3[:],
                       start=True, stop=False, skip_group_check=True)
                    # c0 accumulates modal's out0 region, cols 352:480
                    mm(ps_o[:, 352:480], xA[:, a0:a0 + 128], w0[:],
                       start=False, stop=True, skip_group_check=True)
                    mm(ps_o[:, 0:128], xA[:, a0 + 128:a0 + 256], w1[:],
                       start=True, stop=True, skip_group_check=True)
                    mm(ps_o[:, 128:256], xA[:, a0 + 256:a0 + 384], w2[:],
                       start=True, stop=True, skip_group_check=True)

                    o_sl = out_sb[:, b * DTOT:(b + 1) * DTOT]
                    if b % 2 == 0:
                        nc.vector.tensor_copy(o_sl, ps_o[:])
                    else:
                        nc.scalar.copy(o_sl, ps_o[:])

                for lo, hi in ((0, h), (h, sblk)):
                    if hi <= lo:
                        continue
                    nc.sync.dma_start(
                        out=ys[:, (b0 + lo) * DTOT:(b0 + hi) * DTOT],
                        in_=out_sb[:, lo * DTOT:hi * DTOT])

    nc.compile()
    return nc


_NC_CACHE = {}


def kernel(x, modal_attr, W0, W0m, W1, W2, batch):
    in_maps, ns, ns_pad = _host_prep(x, modal_attr, W0, W0m, W1, W2, batch)
    if ns_pad not in _NC_CACHE:
        _NC_CACHE[ns_pad] = _build_nc(ns_pad)
    nc = _NC_CACHE[ns_pad]
    res = run_bass_kernel_spmd(nc, in_maps, core_ids=list(range(N_CORES)))
    nblk = ns_pad // BLK
    pout = _out_perm()
    invp = np.empty(DTOT, dtype=np.int64)
    invp[pout] = np.arange(DTOT)
    outs = []
    for i in range(N_CORES):
        ysd = res.results[i]["ys"].reshape(128, nblk, DTOT)
        o = ysd[:, :, invp].transpose(1, 0, 2).reshape(ns_pad, DTOT)[:ns]
        outs.append(o.astype(np.float32))
    return np.ascontiguousarray(np.concatenate(outs, axis=0))
